# revision 11
# baseline (speedup 1.0000x reference)
"""Causal self-attention (B=4, T=2048, E=1024, H=16, D=64) on 8 TRN2 NeuronCores.

Sharding: core c -> batch b=c//2, head-group g=c%2 (8 heads each).
Each core:
  phase A: qT,kT = (Wq_g @ x_b.T), (Wk_g @ x_b.T)      feature-major [512, 2048]
  phase B: v = x_b @ Wv_g.T                             token-major, +ones column
  phase C: per head: S^T = k^T q (keys on partitions), exp via ScalarE,
           causal mask as 0/1 multiply, AV matmul with ones-column folding the
           softmax denominator into partition 64, normalize via reciprocal +
           gpsimd partition_broadcast.
  phase D: partial out = concat(heads_g) @ Wp_g.T       token-major [2048, 1024]
Host: out[b] = partial[2b] + partial[2b+1] + bp.

All matmuls run in float32r (full PE rate, ~1e-4 precision); fp32 elsewhere.
"""
import sys

if '/opt/trn_rl_repo' not in sys.path:
    sys.path.insert(0, '/opt/trn_rl_repo')

from contextlib import ExitStack

import numpy as np
import ml_dtypes

import concourse.bass as bass
import concourse.tile as tile
from concourse import bacc, mybir
from concourse.bass_utils import run_bass_kernel_spmd

P = 128
T = 2048          # tokens per core (one batch)
E = 1024          # embed
HPC = 8           # heads per core
D = 64            # head dim
GD = HPC * D      # 512 group dims per core
NB = T // 512     # 4 query/token blocks of 512
NE = E // P       # 8 contraction chunks
NT = T // P       # 16 token blocks of 128
SLOT = D + 1      # v slot width: [v(64) | ones]
F32 = mybir.dt.float32
FPR = mybir.dt.float32r
EXP = mybir.ActivationFunctionType.Exp

_CACHE = {}


def _build():
    nc = bacc.Bacc("TRN2", target_bir_lowering=False, debug=False)

    xt = nc.dram_tensor("xt", [E, T], FPR, kind="ExternalInput").ap()
    wq = nc.dram_tensor("wq", [E, GD], FPR, kind="ExternalInput").ap()
    wk = nc.dram_tensor("wk", [E, GD], FPR, kind="ExternalInput").ap()
    wv = nc.dram_tensor("wv", [E, GD], FPR, kind="ExternalInput").ap()
    wp = nc.dram_tensor("wp", [GD, E], FPR, kind="ExternalInput").ap()
    bqk = nc.dram_tensor("bqk", [2, P, 4], F32, kind="ExternalInput").ap()
    bvb = nc.dram_tensor("bvb", [P, GD], F32, kind="ExternalInput").ap()
    masks = nc.dram_tensor("masks", [4, P, 512], mybir.dt.bfloat16, kind="ExternalInput").ap()
    ones = nc.dram_tensor("ones", [P, HPC], FPR, kind="ExternalInput").ap()
    out = nc.dram_tensor("out", [T, E], F32, kind="ExternalOutput").ap()

    xtr = xt.rearrange("(eo ep) t -> ep eo t", ep=P)      # [128, 8, 2048]
    wqr = wq.rearrange("(eo ep) m -> ep eo m", ep=P)      # [128, 8, 512]
    wkr = wk.rearrange("(eo ep) m -> ep eo m", ep=P)
    wvr = wv.rearrange("(eo ep) m -> ep eo m", ep=P)
    wpr = wp.rearrange("(co cp) o -> cp co o", cp=P)      # [128, 4, 1024]
    outr = out.rearrange("(to tp) o -> to tp o", tp=P)    # [16, 128, 1024]

    with tile.TileContext(nc) as tc, ExitStack() as ctx:
        # ---- persistent pools ----
        qk_pool = ctx.enter_context(tc.tile_pool(name="qk", bufs=1))
        vaug_pool = ctx.enter_context(tc.tile_pool(name="vaug", bufs=1))
        ho_pool = ctx.enter_context(tc.tile_pool(name="ho", bufs=1))
        const_pool = ctx.enter_context(tc.tile_pool(name="const", bufs=1))
        exp_pool = ctx.enter_context(tc.tile_pool(name="exps", bufs=3))
        norm_pool = ctx.enter_context(tc.tile_pool(name="norm", bufs=1))
        out_pool = ctx.enter_context(tc.tile_pool(name="outs", bufs=2))

        qT = [qk_pool.tile([P, T], FPR, tag=f"qT{i}", name=f"qT{i}") for i in range(4)]
        kT = [qk_pool.tile([P, T], FPR, tag=f"kT{i}", name=f"kT{i}") for i in range(4)]
        vaug = [vaug_pool.tile([P, HPC * SLOT], FPR, tag=f"va{t}", name=f"va{t}")
                for t in range(NT)]
        hoT = [ho_pool.tile([P, T], FPR, tag=f"hoT{i}", name=f"hoT{i}") for i in range(4)]

        bqk_t = const_pool.tile([P, 2, 4], F32, tag="bqk", name="bqk_t")
        nc.sync.dma_start(bqk_t[:], bqk.rearrange("b p m -> p b m"))
        bvb_t = const_pool.tile([P, GD], F32, tag="bvb", name="bvb_t")
        nc.sync.dma_start(bvb_t[:], bvb)
        masks_t = const_pool.tile([P, 4, 512], mybir.dt.bfloat16, tag="masks", name="masks_t")
        nc.sync.dma_start(masks_t[:], masks.rearrange("j p n -> p j n"))

        # ---- phase A: q,k projections (feature-major) ----
        with ExitStack() as actx:
            wqk_pool = actx.enter_context(tc.tile_pool(name="wqk", bufs=1))
            xs_pool = actx.enter_context(tc.tile_pool(name="xs", bufs=2))
            psA = actx.enter_context(tc.tile_pool(name="psA", bufs=4, space="PSUM"))

            wq_t = wqk_pool.tile([P, NE, GD], FPR, tag="wq", name="wq_t")
            nc.sync.dma_start(wq_t[:], wqr)
            wk_t = wqk_pool.tile([P, NE, GD], FPR, tag="wk", name="wk_t")
            nc.sync.dma_start(wk_t[:], wkr)

            for n in range(NB):
                nsl = bass.ts(n, 512)
                qps = [psA.tile([P, 512], F32, tag="qps", name=f"qps{n}_{m}")
                       for m in range(4)]
                kps = [psA.tile([P, 512], F32, tag="kps", name=f"kps{n}_{m}")
                       for m in range(4)]
                for eh in range(NE // 2):
                    xs = xs_pool.tile([P, 2, 512], FPR, tag="xs", name=f"xsA{n}_{eh}")
                    nc.sync.dma_start(xs[:], xtr[:, 2 * eh:2 * eh + 2, nsl])
                    for i in range(2):
                        e = 2 * eh + i
                        st = (e == 0)
                        sp = (e == NE - 1)
                        xmov = xs[:, i]
                        for m in range(4):
                            nc.tensor.matmul(qps[m][:], wq_t[:, e, bass.ts(m, P)],
                                             xmov, start=st, stop=sp)
                        for m in range(4):
                            nc.tensor.matmul(kps[m][:], wk_t[:, e, bass.ts(m, P)],
                                             xmov, start=st, stop=sp)
                for m in range(4):
                    nc.vector.tensor_scalar_add(qT[m][:, nsl], qps[m][:], bqk_t[:, 0, m:m + 1])
                for m in range(4):
                    nc.vector.tensor_scalar_add(kT[m][:, nsl], kps[m][:], bqk_t[:, 1, m:m + 1])

        # ---- phase B: v projection (token-major, with ones column) ----
        with ExitStack() as bctx:
            wv_pool = bctx.enter_context(tc.tile_pool(name="wvp", bufs=1))
            xb_pool = bctx.enter_context(tc.tile_pool(name="xb", bufs=2))
            psB = bctx.enter_context(tc.tile_pool(name="psB", bufs=2, space="PSUM"))

            wv_t = wv_pool.tile([P, NE, GD], FPR, tag="wv", name="wv_t")
            nc.sync.dma_start(wv_t[:], wvr)

            for t in range(NT):
                xb = xb_pool.tile([P, NE, P], FPR, tag="xb", name=f"xb{t}")
                nc.sync.dma_start(xb[:], xtr[:, :, bass.ts(t, P)])
                vps = psB.tile([P, GD], F32, tag="vps", name=f"vps{t}")
                for e in range(NE):
                    nc.tensor.matmul(vps[:], xb[:, e], wv_t[:, e],
                                     start=(e == 0), stop=(e == NE - 1))
                nc.sync.dma_start(
                    vaug[t][:].rearrange("p (h s) -> p h s", s=SLOT)[:, :, D:D + 1],
                    ones.unsqueeze(2))
                nc.vector.tensor_tensor(
                    vaug[t][:].rearrange("p (h s) -> p h s", s=SLOT)[:, :, 0:D],
                    vps[:].rearrange("p (h d) -> p h d", d=D),
                    bvb_t[:].rearrange("p (h d) -> p h d", d=D),
                    mybir.AluOpType.add)

        # ---- phase C: attention per head-pair ----
        with ExitStack() as cctx:
            psC = cctx.enter_context(tc.tile_pool(name="psC", bufs=3, space="PSUM"))
            psAV = cctx.enter_context(tc.tile_pool(name="psAV", bufs=1, space="PSUM"))

            for hp in range(4):
                slotA = bass.ds((2 * hp) * SLOT, SLOT)
                slotB = bass.ds((2 * hp + 1) * SLOT, SLOT)
                for qb in range(NB):
                    qsl = bass.ts(qb, 512)
                    nkb = 4 * (qb + 1)
                    avA = psAV.tile([P, 512], F32, tag="avA", name=f"avA{hp}_{qb}")
                    avB = psAV.tile([P, 512], F32, tag="avB", name=f"avB{hp}_{qb}")

                    def emit_s(kb, hp=hp, qb=qb, qsl=qsl):
                        ksl = bass.ts(kb, P)
                        sp = psC.tile([P, 1024], F32, tag="sps", name=f"sps{hp}_{qb}_{kb}")
                        nc.tensor.matmul(sp[:, 0:512], kT[hp][0:D, ksl],
                                         qT[hp][0:D, qsl],
                                         start=True, stop=True)
                        nc.tensor.matmul(sp[:, 512:1024], kT[hp][D:P, ksl],
                                         qT[hp][D:P, qsl],
                                         start=True, stop=True)
                        et = exp_pool.tile([P, 1024], FPR, tag="expS", name=f"ex{hp}_{qb}_{kb}")
                        nc.scalar.activation(et[:], sp[:], EXP, scale=0.125)
                        j = kb - 4 * qb
                        if j >= 0:
                            nc.vector.tensor_tensor(
                                et[:].rearrange("p (two n) -> p two n", two=2),
                                et[:].rearrange("p (two n) -> p two n", two=2),
                                masks_t[:, j:j + 1, :].to_broadcast([P, 2, 512]),
                                mybir.AluOpType.mult)
                        return et

                    pend = {0: emit_s(0)}
                    for kb in range(nkb):
                        if kb + 1 < nkb:
                            pend[kb + 1] = emit_s(kb + 1)
                        et = pend.pop(kb)
                        st = (kb == 0)
                        sp_ = (kb == nkb - 1)
                        nc.tensor.matmul(avA[0:SLOT, :], vaug[kb][:, slotA],
                                         et[:, 0:512], start=st, stop=sp_)
                        nc.tensor.matmul(avB[0:SLOT, :], vaug[kb][:, slotB],
                                         et[:, 512:1024], start=st, stop=sp_)

                    # normalize: out[d, q] * (1 / denom[q]); denom at partition 64
                    # (ones column is last in each v slot). DVE needs 32-aligned
                    # partition bases; partition_broadcast reads physical partition 0,
                    # so shift the reciprocal row 64 -> 0 with a tiny DMA first.
                    rcA = norm_pool.tile([SLOT, 512], F32, tag="rcA", name=f"rcA{hp}_{qb}")
                    nc.vector.reciprocal(rcA[D:SLOT, :], avA[D:SLOT, :])
                    rcB = norm_pool.tile([SLOT, 512], F32, tag="rcB", name=f"rcB{hp}_{qb}")
                    nc.vector.reciprocal(rcB[D:SLOT, :], avB[D:SLOT, :])
                    rc0A = norm_pool.tile([1, 512], F32, tag="rc0A", name=f"rc0A{hp}_{qb}")
                    nc.sync.dma_start(rc0A[0:1, :], rcA[D:SLOT, :])
                    rc0B = norm_pool.tile([1, 512], F32, tag="rc0B", name=f"rc0B{hp}_{qb}")
                    nc.sync.dma_start(rc0B[0:1, :], rcB[D:SLOT, :])
                    bcA = norm_pool.tile([D, 512], F32, tag="bcA", name=f"bcA{hp}_{qb}")
                    nc.gpsimd.partition_broadcast(bcA[:], rc0A[0:1, :], channels=D)
                    bcB = norm_pool.tile([D, 512], F32, tag="bcB", name=f"bcB{hp}_{qb}")
                    nc.gpsimd.partition_broadcast(bcB[:], rc0B[0:1, :], channels=D)
                    nc.vector.tensor_tensor(hoT[hp][0:D, qsl], avA[0:D, :], bcA[:],
                                            mybir.AluOpType.mult)
                    tmpB = norm_pool.tile([D, 512], FPR, tag="tmpB", name=f"tmpB{hp}_{qb}")
                    nc.vector.tensor_tensor(tmpB[:], avB[0:D, :], bcB[:],
                                            mybir.AluOpType.mult)
                    nc.sync.dma_start(hoT[hp][D:P, qsl], tmpB[:])

        # ---- phase D: output projection (token-major partial) ----
        with ExitStack() as dctx:
            wp_pool = dctx.enter_context(tc.tile_pool(name="wpp", bufs=1))
            psD = dctx.enter_context(tc.tile_pool(name="psD", bufs=2, space="PSUM"))
            wp_t = wp_pool.tile([P, 4, E], FPR, tag="wp", name="wp_t")
            nc.sync.dma_start(wp_t[:], wpr)

            for t in range(NT):
                tsl = bass.ts(t, P)
                for nh in range(2):
                    dps = psD.tile([P, 512], F32, tag="dps", name=f"dps{t}_{nh}")
                    for c in range(4):
                        nc.tensor.matmul(dps[:], hoT[c][:, tsl],
                                         wp_t[:, c, bass.ts(nh, 512)],
                                         start=(c == 0), stop=(c == 3))
                    ot = out_pool.tile([P, 512], F32, tag="ot", name=f"ot{t}_{nh}")
                    nc.vector.tensor_copy(ot[:], dps[:])
                    nc.sync.dma_start(outr[t, :, bass.ts(nh, 512)], ot[:])

    nc.compile()
    return nc


def _in_maps(x, Wq, bq, Wk, bk, Wv, bv, Wp, bp):
    maskv = np.zeros((4, P, 512), np.float32)
    for j in range(4):
        kidx = 128 * j + np.arange(P)[:, None]
        maskv[j] = (kidx <= np.arange(512)[None, :]).astype(np.float32)
    maps = []
    for c in range(8):
        b, g = divmod(c, 2)
        gs = slice(512 * g, 512 * (g + 1))
        maps.append({
            "xt": np.ascontiguousarray(x[b].T),
            "wq": np.ascontiguousarray(Wq[gs, :].T),
            "wk": np.ascontiguousarray(Wk[gs, :].T),
            "wv": np.ascontiguousarray(Wv[gs, :].T),
            "wp": np.ascontiguousarray(Wp[:, gs].T),
            "bqk": np.stack([bq[gs].reshape(4, P).T, bk[gs].reshape(4, P).T]),
            "bvb": np.broadcast_to(bv[gs], (P, 512)).copy(),
            "masks": maskv.astype(ml_dtypes.bfloat16),
            "ones": np.ones((P, HPC), np.float32),
        })
    return maps


def kernel(x, Wq, bq, Wk, bk, Wv, bv, Wp, bp, _trace=False):
    if "nc" not in _CACHE:
        _CACHE["nc"] = _build()
    nc = _CACHE["nc"]
    res = run_bass_kernel_spmd(nc, _in_maps(x, Wq, bq, Wk, bk, Wv, bv, Wp, bp),
                               list(range(8)), trace=_trace)
    _CACHE["last_result"] = res
    out = np.empty((4, T, E), np.float32)
    for b in range(4):
        out[b] = res.results[2 * b]["out"] + res.results[2 * b + 1]["out"] + bp
    return out


# revision 16
# speedup vs baseline: 1.2466x; 1.2466x over previous
"""Causal self-attention (B=4, T=2048, E=1024, H=16, D=64) on 8 TRN2 NeuronCores.

Sharding: core c -> batch b=c//2, head-group g=c%2 (8 heads each).
Each core:
  phase A: qT,kT = (Wq_g @ x_b.T), (Wk_g @ x_b.T)      feature-major [512, 2048]
  phase B: v = x_b @ Wv_g.T                             token-major, +ones column
  phase C: per head: S^T = k^T q (keys on partitions), exp via ScalarE,
           causal mask as 0/1 multiply, AV matmul with ones-column folding the
           softmax denominator into partition 64, normalize via reciprocal +
           gpsimd partition_broadcast.
  phase D: partial out = concat(heads_g) @ Wp_g.T       token-major [2048, 1024]
Host: out[b] = partial[2b] + partial[2b+1] + bp.

All matmuls run in float32r (full PE rate, ~1e-4 precision); fp32 elsewhere.
"""
import sys

if '/opt/trn_rl_repo' not in sys.path:
    sys.path.insert(0, '/opt/trn_rl_repo')

from contextlib import ExitStack

import numpy as np
import ml_dtypes

import concourse.bass as bass
import concourse.tile as tile
from concourse import bacc, mybir
from concourse.bass_utils import run_bass_kernel_spmd

P = 128
T = 2048          # tokens per core (one batch)
E = 1024          # embed
HPC = 8           # heads per core
D = 64            # head dim
GD = HPC * D      # 512 group dims per core
NB = T // 512     # 4 query/token blocks of 512
NE = E // P       # 8 contraction chunks
NT = T // P       # 16 token blocks of 128
SLOT = D + 1      # v slot width: [v(64) | ones]
F32 = mybir.dt.float32
FPR = mybir.dt.float32r
EXP = mybir.ActivationFunctionType.Exp

_CACHE = {}


def _build():
    nc = bacc.Bacc("TRN2", target_bir_lowering=False, debug=False)

    xt = nc.dram_tensor("xt", [E, T], FPR, kind="ExternalInput").ap()
    wq = nc.dram_tensor("wq", [E, GD], FPR, kind="ExternalInput").ap()
    wk = nc.dram_tensor("wk", [E, GD], FPR, kind="ExternalInput").ap()
    wv = nc.dram_tensor("wv", [E, GD], FPR, kind="ExternalInput").ap()
    wp = nc.dram_tensor("wp", [GD, E], FPR, kind="ExternalInput").ap()
    bqk = nc.dram_tensor("bqk", [2, P, 4], F32, kind="ExternalInput").ap()
    bvb = nc.dram_tensor("bvb", [P, GD], F32, kind="ExternalInput").ap()
    masks = nc.dram_tensor("masks", [P, P], mybir.dt.bfloat16, kind="ExternalInput").ap()
    ones = nc.dram_tensor("ones", [P, HPC], FPR, kind="ExternalInput").ap()
    out = nc.dram_tensor("out", [T, E], F32, kind="ExternalOutput").ap()

    xtr = xt.rearrange("(eo ep) t -> ep eo t", ep=P)      # [128, 8, 2048]
    wqr = wq.rearrange("(eo ep) m -> ep eo m", ep=P)      # [128, 8, 512]
    wkr = wk.rearrange("(eo ep) m -> ep eo m", ep=P)
    wvr = wv.rearrange("(eo ep) m -> ep eo m", ep=P)
    wpr = wp.rearrange("(co cp) o -> cp co o", cp=P)      # [128, 4, 1024]
    outr = out.rearrange("(to tp) o -> to tp o", tp=P)    # [16, 128, 1024]

    with tile.TileContext(nc) as tc, ExitStack() as ctx:
        # ---- persistent pools ----
        qk_pool = ctx.enter_context(tc.tile_pool(name="qk", bufs=1))
        vaug_pool = ctx.enter_context(tc.tile_pool(name="vaug", bufs=1))
        ho_pool = ctx.enter_context(tc.tile_pool(name="ho", bufs=1))
        const_pool = ctx.enter_context(tc.tile_pool(name="const", bufs=1))
        exp_pool = ctx.enter_context(tc.tile_pool(name="exps", bufs=3))
        norm_pool = ctx.enter_context(tc.tile_pool(name="norm", bufs=1))
        out_pool = ctx.enter_context(tc.tile_pool(name="outs", bufs=2))

        qT = [qk_pool.tile([P, T], FPR, tag=f"qT{i}", name=f"qT{i}") for i in range(4)]
        kT = [qk_pool.tile([P, T], FPR, tag=f"kT{i}", name=f"kT{i}") for i in range(4)]
        vaug = [vaug_pool.tile([P, HPC * SLOT], FPR, tag=f"va{t}", name=f"va{t}")
                for t in range(NT)]
        hoT = [ho_pool.tile([P, T], FPR, tag=f"hoT{i}", name=f"hoT{i}") for i in range(4)]

        bqk_t = const_pool.tile([P, 2, 4], F32, tag="bqk", name="bqk_t")
        nc.sync.dma_start(bqk_t[:], bqk.rearrange("b p m -> p b m"))
        bvb_t = const_pool.tile([P, GD], F32, tag="bvb", name="bvb_t")
        nc.sync.dma_start(bvb_t[:], bvb)
        masks_t = const_pool.tile([P, P], mybir.dt.bfloat16, tag="masks", name="masks_t")
        nc.sync.dma_start(masks_t[:], masks)
        onec_t = const_pool.tile([P, HPC], FPR, tag="onec", name="onec_t")
        nc.sync.dma_start(onec_t[:], ones)

        # ---- phase A: q,k projections (feature-major) ----
        with ExitStack() as actx:
            wqk_pool = actx.enter_context(tc.tile_pool(name="wqk", bufs=1))
            xs_pool = actx.enter_context(tc.tile_pool(name="xs", bufs=2))
            psA = actx.enter_context(tc.tile_pool(name="psA", bufs=4, space="PSUM"))

            wq_t = wqk_pool.tile([P, NE, GD], FPR, tag="wq", name="wq_t")
            nc.sync.dma_start(wq_t[:], wqr)
            wk_t = wqk_pool.tile([P, NE, GD], FPR, tag="wk", name="wk_t")
            nc.sync.dma_start(wk_t[:], wkr)

            for n in range(NB):
                nsl = bass.ts(n, 512)
                qps = [psA.tile([P, 512], F32, tag="qps", name=f"qps{n}_{m}")
                       for m in range(4)]
                kps = [psA.tile([P, 512], F32, tag="kps", name=f"kps{n}_{m}")
                       for m in range(4)]
                for eh in range(NE // 2):
                    xs = xs_pool.tile([P, 2, 512], FPR, tag="xs", name=f"xsA{n}_{eh}")
                    nc.sync.dma_start(xs[:], xtr[:, 2 * eh:2 * eh + 2, nsl])
                    for i in range(2):
                        e = 2 * eh + i
                        st = (e == 0)
                        sp = (e == NE - 1)
                        xmov = xs[:, i]
                        for m in range(4):
                            nc.tensor.matmul(qps[m][:], wq_t[:, e, bass.ts(m, P)],
                                             xmov, start=st, stop=sp)
                        for m in range(4):
                            nc.tensor.matmul(kps[m][:], wk_t[:, e, bass.ts(m, P)],
                                             xmov, start=st, stop=sp)
                for m in range(4):
                    nc.vector.tensor_scalar_add(qT[m][:, nsl], qps[m][:], bqk_t[:, 0, m:m + 1])
                for m in range(4):
                    nc.vector.tensor_scalar_add(kT[m][:, nsl], kps[m][:], bqk_t[:, 1, m:m + 1])

        # ---- phase B: v projection (token-major, with ones column) ----
        with ExitStack() as bctx:
            wv_pool = bctx.enter_context(tc.tile_pool(name="wvp", bufs=1))
            xb_pool = bctx.enter_context(tc.tile_pool(name="xb", bufs=2))
            psB = bctx.enter_context(tc.tile_pool(name="psB", bufs=4, space="PSUM"))

            wv_t = wv_pool.tile([P, NE, GD], FPR, tag="wv", name="wv_t")
            nc.sync.dma_start(wv_t[:], wvr)

            for tg in range(4):
                vps = [psB.tile([P, GD], F32, tag="vps", name=f"vps{tg}_{jj}")
                       for jj in range(4)]
                for ep in range(4):
                    xbt = xb_pool.tile([P, 2, 512], FPR, tag="xb", name=f"xb{tg}_{ep}")
                    nc.sync.dma_start(xbt[:], xtr[:, 2 * ep:2 * ep + 2, bass.ts(tg, 512)])
                    for i in range(2):
                        e = 2 * ep + i
                        for jj in range(4):
                            nc.tensor.matmul(vps[jj][:], xbt[:, i, bass.ts(jj, P)],
                                             wv_t[:, e],
                                             start=(e == 0), stop=(e == NE - 1))
                for jj in range(4):
                    t = 4 * tg + jj
                    nc.vector.tensor_copy(
                        vaug[t][:].rearrange("p (h s) -> p h s", s=SLOT)[:, :, D:D + 1],
                        onec_t[:].unsqueeze(2))
                    nc.vector.tensor_tensor(
                        vaug[t][:].rearrange("p (h s) -> p h s", s=SLOT)[:, :, 0:D],
                        vps[jj][:].rearrange("p (h d) -> p h d", d=D),
                        bvb_t[:].rearrange("p (h d) -> p h d", d=D),
                        mybir.AluOpType.add)

        # ---- phase C: attention per head-pair ----
        with ExitStack() as cctx:
            psC = cctx.enter_context(tc.tile_pool(name="psC", bufs=2, space="PSUM"))
            psAV = cctx.enter_context(tc.tile_pool(name="psAV", bufs=2, space="PSUM"))

            for hp in range(4):
                slotA = bass.ds((2 * hp) * SLOT, SLOT)
                slotB = bass.ds((2 * hp + 1) * SLOT, SLOT)
                for qb in range(NB):
                    qsl0 = qb * 512
                    nkb = 4 * (qb + 1)
                    avA = psAV.tile([P, 512], F32, tag="avA", name=f"avA{hp}_{qb}")
                    avB = psAV.tile([P, 512], F32, tag="avB", name=f"avB{hp}_{qb}")

                    def emit_s(kb, hp=hp, qb=qb, qsl0=qsl0):
                        # straddle tile j: columns f < 128j are fully masked ->
                        # compute only columns [q0:512]; the triangle mask only
                        # touches the diagonal 128-column block [q0:q0+128).
                        ksl = bass.ts(kb, P)
                        j = kb - 4 * qb
                        q0 = 128 * j if j > 0 else 0
                        sp = psC.tile([P, 1024], F32, tag="sps", name=f"sps{hp}_{qb}_{kb}")
                        et = exp_pool.tile([P, 1024], FPR, tag="expS", name=f"ex{hp}_{qb}_{kb}")
                        for half, pl, ph in ((0, 0, D), (1, D, P)):
                            o = 512 * half
                            nc.tensor.matmul(sp[:, o + q0:o + 512], kT[hp][pl:ph, ksl],
                                             qT[hp][pl:ph, qsl0 + q0:qsl0 + 512],
                                             start=True, stop=True)
                        if j < 0 or q0 == 0:
                            nc.scalar.activation(et[:], sp[:], EXP, scale=0.125)
                        else:
                            nc.scalar.activation(et[:, q0:512], sp[:, q0:512],
                                                 EXP, scale=0.125)
                            nc.scalar.activation(et[:, 512 + q0:1024], sp[:, 512 + q0:1024],
                                                 EXP, scale=0.125)
                        if j >= 0:
                            dsl = bass.ds(q0, P)
                            nc.vector.tensor_tensor(
                                et[:].rearrange("p (two n) -> p two n", two=2)[:, :, dsl],
                                et[:].rearrange("p (two n) -> p two n", two=2)[:, :, dsl],
                                masks_t[:, None, :].to_broadcast([P, 2, P]),
                                mybir.AluOpType.mult)
                        return et, q0

                    pend = {0: emit_s(0)}
                    for kb in range(nkb):
                        if kb + 1 < nkb:
                            pend[kb + 1] = emit_s(kb + 1)
                        et, q0 = pend.pop(kb)
                        st = (kb == 0)
                        sp_ = (kb == nkb - 1)
                        nc.tensor.matmul(avA[0:SLOT, q0:512], vaug[kb][:, slotA],
                                         et[:, q0:512], start=st, stop=sp_)
                        nc.tensor.matmul(avB[0:SLOT, q0:512], vaug[kb][:, slotB],
                                         et[:, 512 + q0:1024], start=st, stop=sp_)

                    # normalize: out[d, q] * (1 / denom[q]); denom at partition 64
                    # (ones column is last in each v slot). Reciprocal on a
                    # [32, 32] reshaped view (32 lanes) instead of [1, 512]
                    # (1 lane); plain DVE reciprocal for exact numerics.
                    # partition_broadcast reads physical partition 0.
                    qsl = bass.ds(qsl0, 512)
                    dnA = norm_pool.tile([SLOT, 512], F32, tag="dnA", name=f"dnA{hp}_{qb}")
                    nc.scalar.copy(dnA[D:SLOT, :], avA[D:SLOT, :])
                    dnB = norm_pool.tile([SLOT, 512], F32, tag="dnB", name=f"dnB{hp}_{qb}")
                    nc.vector.tensor_copy(dnB[D:SLOT, :], avB[D:SLOT, :])
                    d32 = norm_pool.tile([32, 32], F32, tag="d32", name=f"d32{hp}_{qb}")
                    nc.sync.dma_start(d32[:, 0:16], dnA[D:SLOT, :])
                    nc.sync.dma_start(d32[:, 16:32], dnB[D:SLOT, :])
                    r32 = norm_pool.tile([32, 32], F32, tag="r32", name=f"r32{hp}_{qb}")
                    nc.vector.reciprocal(r32[:], d32[:])
                    rc0A = norm_pool.tile([1, 512], F32, tag="rc0A", name=f"rc0A{hp}_{qb}")
                    nc.sync.dma_start(rc0A[0:1, :], r32[:, 0:16])
                    rc0B = norm_pool.tile([1, 512], F32, tag="rc0B", name=f"rc0B{hp}_{qb}")
                    nc.sync.dma_start(rc0B[0:1, :], r32[:, 16:32])
                    bcA = norm_pool.tile([D, 512], F32, tag="bcA", name=f"bcA{hp}_{qb}")
                    nc.gpsimd.partition_broadcast(bcA[:], rc0A[0:1, :], channels=D)
                    bcB = norm_pool.tile([D, 512], F32, tag="bcB", name=f"bcB{hp}_{qb}")
                    nc.gpsimd.partition_broadcast(bcB[:], rc0B[0:1, :], channels=D)
                    nc.vector.tensor_tensor(hoT[hp][0:D, qsl], avA[0:D, :], bcA[:],
                                            mybir.AluOpType.mult)
                    tmpB = norm_pool.tile([D, 512], FPR, tag="tmpB", name=f"tmpB{hp}_{qb}")
                    nc.vector.tensor_tensor(tmpB[:], avB[0:D, :], bcB[:],
                                            mybir.AluOpType.mult)
                    nc.sync.dma_start(hoT[hp][D:P, qsl], tmpB[:])

        # ---- phase D: output projection (token-major partial) ----
        with ExitStack() as dctx:
            wp_pool = dctx.enter_context(tc.tile_pool(name="wpp", bufs=1))
            psD = dctx.enter_context(tc.tile_pool(name="psD", bufs=2, space="PSUM"))
            wp_t = wp_pool.tile([P, 4, E], FPR, tag="wp", name="wp_t")
            nc.sync.dma_start(wp_t[:], wpr)

            for t in range(NT):
                tsl = bass.ts(t, P)
                for nh in range(2):
                    dps = psD.tile([P, 512], F32, tag="dps", name=f"dps{t}_{nh}")
                    for c in range(4):
                        nc.tensor.matmul(dps[:], hoT[c][:, tsl],
                                         wp_t[:, c, bass.ts(nh, 512)],
                                         start=(c == 0), stop=(c == 3))
                    ot = out_pool.tile([P, 512], F32, tag="ot", name=f"ot{t}_{nh}")
                    nc.vector.tensor_copy(ot[:], dps[:])
                    nc.sync.dma_start(outr[t, :, bass.ts(nh, 512)], ot[:])

    nc.compile()
    return nc


def _in_maps(x, Wq, bq, Wk, bk, Wv, bv, Wp, bp):
    maskv = (np.arange(P)[:, None] <= np.arange(P)[None, :]).astype(np.float32)
    maps = []
    for c in range(8):
        b, g = divmod(c, 2)
        gs = slice(512 * g, 512 * (g + 1))
        maps.append({
            "xt": np.ascontiguousarray(x[b].T),
            "wq": np.ascontiguousarray(Wq[gs, :].T),
            "wk": np.ascontiguousarray(Wk[gs, :].T),
            "wv": np.ascontiguousarray(Wv[gs, :].T),
            "wp": np.ascontiguousarray(Wp[:, gs].T),
            "bqk": np.stack([bq[gs].reshape(4, P).T, bk[gs].reshape(4, P).T]),
            "bvb": np.broadcast_to(bv[gs], (P, 512)).copy(),
            "masks": maskv.astype(ml_dtypes.bfloat16),
            "ones": np.ones((P, HPC), np.float32),
        })
    return maps


def kernel(x, Wq, bq, Wk, bk, Wv, bv, Wp, bp, _trace=False):
    if "nc" not in _CACHE:
        _CACHE["nc"] = _build()
    nc = _CACHE["nc"]
    res = run_bass_kernel_spmd(nc, _in_maps(x, Wq, bq, Wk, bk, Wv, bv, Wp, bp),
                               list(range(8)), trace=_trace)
    _CACHE["last_result"] = res
    out = np.empty((4, T, E), np.float32)
    for b in range(4):
        out[b] = res.results[2 * b]["out"] + res.results[2 * b + 1]["out"] + bp
    return out


# revision 20
# speedup vs baseline: 1.3087x; 1.0499x over previous
"""Causal self-attention (B=4, T=2048, E=1024, H=16, D=64) on 8 TRN2 NeuronCores.

Sharding: core c -> batch b=c//2, head-group g=c%2 (8 heads each).
Each core:
  phase A: qT,kT = (Wq_g @ x_b.T), (Wk_g @ x_b.T)      feature-major [512, 2048]
  phase B: v = x_b @ Wv_g.T                             token-major, +ones column
  phase C: per head: S^T = k^T q (keys on partitions), exp via ScalarE,
           causal mask as 0/1 multiply, AV matmul with ones-column folding the
           softmax denominator into partition 64, normalize via reciprocal +
           gpsimd partition_broadcast.
  phase D: partial out = concat(heads_g) @ Wp_g.T       token-major [2048, 1024]
Host: out[b] = partial[2b] + partial[2b+1] + bp.

All matmuls run in float32r (full PE rate, ~1e-4 precision); fp32 elsewhere.
"""
import sys

if '/opt/trn_rl_repo' not in sys.path:
    sys.path.insert(0, '/opt/trn_rl_repo')

from contextlib import ExitStack

import numpy as np
import ml_dtypes

import concourse.bass as bass
import concourse.tile as tile
from concourse import bacc, mybir
from concourse.bass_utils import run_bass_kernel_spmd

P = 128
T = 2048          # tokens per core (one batch)
E = 1024          # embed
HPC = 8           # heads per core
D = 64            # head dim
GD = HPC * D      # 512 group dims per core
NB = T // 512     # 4 query/token blocks of 512
NE = E // P       # 8 contraction chunks
NT = T // P       # 16 token blocks of 128
SLOT = D + 1      # v slot width: [v(64) | ones]
F32 = mybir.dt.float32
FPR = mybir.dt.float32r
EXP = mybir.ActivationFunctionType.Exp

_CACHE = {}


def _build():
    nc = bacc.Bacc("TRN2", target_bir_lowering=False, debug=False)

    xt = nc.dram_tensor("xt", [E, T], FPR, kind="ExternalInput").ap()
    wq = nc.dram_tensor("wq", [E, GD], FPR, kind="ExternalInput").ap()
    wk = nc.dram_tensor("wk", [E, GD], FPR, kind="ExternalInput").ap()
    wv = nc.dram_tensor("wv", [E, GD], FPR, kind="ExternalInput").ap()
    wp = nc.dram_tensor("wp", [GD, E], FPR, kind="ExternalInput").ap()
    bqk = nc.dram_tensor("bqk", [2, P, 4], F32, kind="ExternalInput").ap()
    bvb = nc.dram_tensor("bvb", [P, GD], mybir.dt.bfloat16, kind="ExternalInput").ap()
    masks = nc.dram_tensor("masks", [P, P], mybir.dt.bfloat16, kind="ExternalInput").ap()
    ones = nc.dram_tensor("ones", [P, HPC], FPR, kind="ExternalInput").ap()
    out = nc.dram_tensor("out", [T, E], F32, kind="ExternalOutput").ap()

    xtr = xt.rearrange("(eo ep) t -> ep eo t", ep=P)      # [128, 8, 2048]
    wqr = wq.rearrange("(eo ep) m -> ep eo m", ep=P)      # [128, 8, 512]
    wkr = wk.rearrange("(eo ep) m -> ep eo m", ep=P)
    wvr = wv.rearrange("(eo ep) m -> ep eo m", ep=P)
    wpr = wp.rearrange("(co cp) o -> cp co o", cp=P)      # [128, 4, 1024]
    outr = out.rearrange("(to tp) o -> to tp o", tp=P)    # [16, 128, 1024]

    with tile.TileContext(nc) as tc, ExitStack() as ctx:
        # ---- persistent pools ----
        qk_pool = ctx.enter_context(tc.tile_pool(name="qk", bufs=1))
        vaug_pool = ctx.enter_context(tc.tile_pool(name="vaug", bufs=1))
        ho_pool = ctx.enter_context(tc.tile_pool(name="ho", bufs=1))
        const_pool = ctx.enter_context(tc.tile_pool(name="const", bufs=1))
        exp_pool = ctx.enter_context(tc.tile_pool(name="exps", bufs=3))
        norm_pool = ctx.enter_context(tc.tile_pool(name="norm", bufs=1))
        out_pool = ctx.enter_context(tc.tile_pool(name="outs", bufs=2))

        qT = [qk_pool.tile([P, T], FPR, tag=f"qT{i}", name=f"qT{i}") for i in range(4)]
        kT = [qk_pool.tile([P, T], FPR, tag=f"kT{i}", name=f"kT{i}") for i in range(4)]
        vaug = [vaug_pool.tile([P, HPC * SLOT], FPR, tag=f"va{t}", name=f"va{t}")
                for t in range(NT)]
        hoT = [ho_pool.tile([P, T], FPR, tag=f"hoT{i}", name=f"hoT{i}") for i in range(4)]

        bqk_t = const_pool.tile([P, 2, 4], F32, tag="bqk", name="bqk_t")
        nc.sync.dma_start(bqk_t[:], bqk.rearrange("b p m -> p b m"))
        bvb_t = const_pool.tile([P, GD], mybir.dt.bfloat16, tag="bvb", name="bvb_t")
        nc.sync.dma_start(bvb_t[:], bvb)
        masks_t = const_pool.tile([P, P], mybir.dt.bfloat16, tag="masks", name="masks_t")
        nc.sync.dma_start(masks_t[:], masks)
        onec_t = const_pool.tile([P, HPC], FPR, tag="onec", name="onec_t")
        nc.sync.dma_start(onec_t[:], ones)

        # ---- phases A+B: q,k,v projections (shared psum slots, scoped weights) ----
        with ExitStack() as actx:
            xs_pool = actx.enter_context(tc.tile_pool(name="xs", bufs=2))
            psA = actx.enter_context(tc.tile_pool(name="psA", bufs=4, space="PSUM"))
            wqk_ctx = ExitStack()
            wqk_pool = wqk_ctx.enter_context(tc.tile_pool(name="wqk", bufs=1))

            wq_t = wqk_pool.tile([P, NE, GD], FPR, tag="wq", name="wq_t")
            wk_t = wqk_pool.tile([P, NE, GD], FPR, tag="wk", name="wk_t")
            for e in range(NE):
                nc.sync.dma_start(wq_t[:, e], wqr[:, e])
                nc.sync.dma_start(wk_t[:, e], wkr[:, e])

            for n in range(NB):
                nsl = bass.ts(n, 512)
                qps = [psA.tile([P, 512], F32, tag="qps", name=f"qps{n}_{m}")
                       for m in range(4)]
                kps = [psA.tile([P, 512], F32, tag="kps", name=f"kps{n}_{m}")
                       for m in range(4)]
                for eh in range(NE // 2):
                    xs = xs_pool.tile([P, 2, 512], FPR, tag="xs", name=f"xsA{n}_{eh}")
                    nc.sync.dma_start(xs[:], xtr[:, 2 * eh:2 * eh + 2, nsl])
                    for i in range(2):
                        e = 2 * eh + i
                        st = (e == 0)
                        sp = (e == NE - 1)
                        xmov = xs[:, i]
                        for m in range(4):
                            nc.tensor.matmul(qps[m][:], wq_t[:, e, bass.ts(m, P)],
                                             xmov, start=st, stop=sp)
                        for m in range(4):
                            nc.tensor.matmul(kps[m][:], wk_t[:, e, bass.ts(m, P)],
                                             xmov, start=st, stop=sp)
                for m in range(4):
                    nc.vector.tensor_scalar_add(qT[m][:, nsl], qps[m][:], bqk_t[:, 0, m:m + 1])
                for m in range(4):
                    nc.vector.tensor_scalar_add(kT[m][:, nsl], kps[m][:], bqk_t[:, 1, m:m + 1])

            # ---- phase B: v projection (token-major, with ones column) ----
            wqk_ctx.close()
            wv_pool = actx.enter_context(tc.tile_pool(name="wvp", bufs=1))
            wv_t = wv_pool.tile([P, NE, GD], FPR, tag="wv", name="wv_t")
            for e in range(NE):
                nc.sync.dma_start(wv_t[:, e], wvr[:, e])
            for tg in range(4):
                vps = [psA.tile([P, GD], F32, tag=("qps" if jj < 2 else "kps"),
                                name=f"vps{tg}_{jj}")
                       for jj in range(4)]
                for ep in range(4):
                    xbt = xs_pool.tile([P, 2, 512], FPR, tag="xs", name=f"xb{tg}_{ep}")
                    nc.sync.dma_start(xbt[:], xtr[:, 2 * ep:2 * ep + 2, bass.ts(tg, 512)])
                    for i in range(2):
                        e = 2 * ep + i
                        for jj in range(4):
                            nc.tensor.matmul(vps[jj][:], xbt[:, i, bass.ts(jj, P)],
                                             wv_t[:, e],
                                             start=(e == 0), stop=(e == NE - 1))
                for jj in range(4):
                    t = 4 * tg + jj
                    nc.vector.tensor_copy(
                        vaug[t][:].rearrange("p (h s) -> p h s", s=SLOT)[:, :, D:D + 1],
                        onec_t[:].unsqueeze(2))
                    nc.vector.tensor_tensor(
                        vaug[t][:].rearrange("p (h s) -> p h s", s=SLOT)[:, :, 0:D],
                        vps[jj][:].rearrange("p (h d) -> p h d", d=D),
                        bvb_t[:].rearrange("p (h d) -> p h d", d=D),
                        mybir.AluOpType.add)

        # ---- phase C: attention per head-pair ----
        with ExitStack() as cctx:
            psC = cctx.enter_context(tc.tile_pool(name="psC", bufs=3, space="PSUM"))
            psAV = cctx.enter_context(tc.tile_pool(name="psAV", bufs=1, space="PSUM"))

            for hp in range(4):
                slotA = bass.ds((2 * hp) * SLOT, SLOT)
                slotB = bass.ds((2 * hp + 1) * SLOT, SLOT)
                for qb in range(NB):
                    qsl0 = qb * 512
                    nkb = 4 * (qb + 1)
                    avA = psAV.tile([P, 512], F32, tag="avA", name=f"avA{hp}_{qb}")
                    avB = psAV.tile([P, 512], F32, tag="avB", name=f"avB{hp}_{qb}")

                    def emit_s(kb, hp=hp, qb=qb, qsl0=qsl0):
                        # straddle tile j: columns f < 128j are fully masked ->
                        # compute only columns [q0:512]; the triangle mask only
                        # touches the diagonal 128-column block [q0:q0+128).
                        ksl = bass.ts(kb, P)
                        j = kb - 4 * qb
                        q0 = 128 * j if j > 0 else 0
                        sp = psC.tile([P, 1024], F32, tag="sps", name=f"sps{hp}_{qb}_{kb}")
                        et = exp_pool.tile([P, 1024], FPR, tag="expS", name=f"ex{hp}_{qb}_{kb}")
                        for half, pl, ph in ((0, 0, D), (1, D, P)):
                            o = 512 * half
                            nc.tensor.matmul(sp[:, o + q0:o + 512], kT[hp][pl:ph, ksl],
                                             qT[hp][pl:ph, qsl0 + q0:qsl0 + 512],
                                             start=True, stop=True)
                        if j < 0 or q0 == 0:
                            nc.scalar.activation(et[:], sp[:], EXP, scale=0.125)
                        else:
                            nc.scalar.activation(et[:, q0:512], sp[:, q0:512],
                                                 EXP, scale=0.125)
                            nc.scalar.activation(et[:, 512 + q0:1024], sp[:, 512 + q0:1024],
                                                 EXP, scale=0.125)
                        if j >= 0:
                            dsl = bass.ds(q0, P)
                            nc.vector.tensor_tensor(
                                et[:].rearrange("p (two n) -> p two n", two=2)[:, :, dsl],
                                et[:].rearrange("p (two n) -> p two n", two=2)[:, :, dsl],
                                masks_t[:, None, :].to_broadcast([P, 2, P]),
                                mybir.AluOpType.mult)
                        return et, q0

                    pend = {0: emit_s(0)}
                    for kb in range(nkb):
                        if kb + 1 < nkb:
                            pend[kb + 1] = emit_s(kb + 1)
                        et, q0 = pend.pop(kb)
                        st = (kb == 0)
                        sp_ = (kb == nkb - 1)
                        nc.tensor.matmul(avA[0:SLOT, q0:512], vaug[kb][:, slotA],
                                         et[:, q0:512], start=st, stop=sp_)
                        nc.tensor.matmul(avB[0:SLOT, q0:512], vaug[kb][:, slotB],
                                         et[:, 512 + q0:1024], start=st, stop=sp_)

                    # normalize: out[d,q] / denom[q]; denom at partition 64 (ones
                    # col last in v slots). Evacuate av psum to SBUF immediately so
                    # the psum bank frees fast (keeps PE dense / HAM warm), then
                    # reciprocal on a [32,32] reshape (32 lanes), broadcast from
                    # physical partition 0, multiply.
                    qsl = bass.ds(qsl0, 512)
                    avSA = norm_pool.tile([SLOT, 512], F32, tag="avSA", name=f"avSA{hp}_{qb}")
                    nc.vector.tensor_copy(avSA[:], avA[0:SLOT, :])
                    avSB = norm_pool.tile([SLOT, 512], F32, tag="avSB", name=f"avSB{hp}_{qb}")
                    nc.vector.tensor_copy(avSB[:], avB[0:SLOT, :])
                    d32 = norm_pool.tile([32, 32], F32, tag="d32", name=f"d32{hp}_{qb}")
                    nc.sync.dma_start(d32[:, 0:16], avSA[D:SLOT, :])
                    nc.sync.dma_start(d32[:, 16:32], avSB[D:SLOT, :])
                    nc.vector.reciprocal(d32[:], d32[:])
                    rc0A = norm_pool.tile([1, 512], F32, tag="rc0A", name=f"rc0A{hp}_{qb}")
                    nc.sync.dma_start(rc0A[0:1, :], d32[:, 0:16])
                    rc0B = norm_pool.tile([1, 512], F32, tag="rc0B", name=f"rc0B{hp}_{qb}")
                    nc.sync.dma_start(rc0B[0:1, :], d32[:, 16:32])
                    bcA = norm_pool.tile([D, 512], F32, tag="bcA", name=f"bcA{hp}_{qb}")
                    nc.gpsimd.partition_broadcast(bcA[:], rc0A[0:1, :], channels=D)
                    bcB = norm_pool.tile([D, 512], F32, tag="bcB", name=f"bcB{hp}_{qb}")
                    nc.gpsimd.partition_broadcast(bcB[:], rc0B[0:1, :], channels=D)
                    nc.vector.tensor_tensor(hoT[hp][0:D, qsl], avSA[0:D, :], bcA[:],
                                            mybir.AluOpType.mult)
                    tmpB = norm_pool.tile([D, 512], FPR, tag="tmpB", name=f"tmpB{hp}_{qb}")
                    nc.vector.tensor_tensor(tmpB[:], avSB[0:D, :], bcB[:],
                                            mybir.AluOpType.mult)
                    nc.sync.dma_start(hoT[hp][D:P, qsl], tmpB[:])

        # ---- phase D: output projection (token-major partial) ----
        with ExitStack() as dctx:
            wp_pool = dctx.enter_context(tc.tile_pool(name="wpp", bufs=1))
            psD = dctx.enter_context(tc.tile_pool(name="psD", bufs=4, space="PSUM"))
            wp_t = wp_pool.tile([P, 4, E], FPR, tag="wp", name="wp_t")
            nc.sync.dma_start(wp_t[:], wpr)

            for t in range(NT):
                tsl = bass.ts(t, P)
                for nh in range(2):
                    dps = psD.tile([P, 512], F32, tag="dps", name=f"dps{t}_{nh}")
                    for c in range(4):
                        nc.tensor.matmul(dps[:], hoT[c][:, tsl],
                                         wp_t[:, c, bass.ts(nh, 512)],
                                         start=(c == 0), stop=(c == 3))
                    ot = out_pool.tile([P, 512], F32, tag="ot", name=f"ot{t}_{nh}")
                    nc.vector.tensor_copy(ot[:], dps[:])
                    nc.sync.dma_start(outr[t, :, bass.ts(nh, 512)], ot[:])

    nc.compile()
    return nc


def _in_maps(x, Wq, bq, Wk, bk, Wv, bv, Wp, bp):
    maskv = (np.arange(P)[:, None] <= np.arange(P)[None, :]).astype(np.float32)
    maps = []
    for c in range(8):
        b, g = divmod(c, 2)
        gs = slice(512 * g, 512 * (g + 1))
        maps.append({
            "xt": np.ascontiguousarray(x[b].T),
            "wq": np.ascontiguousarray(Wq[gs, :].T),
            "wk": np.ascontiguousarray(Wk[gs, :].T),
            "wv": np.ascontiguousarray(Wv[gs, :].T),
            "wp": np.ascontiguousarray(Wp[:, gs].T),
            "bqk": np.stack([bq[gs].reshape(4, P).T, bk[gs].reshape(4, P).T]),
            "bvb": np.broadcast_to(bv[gs], (P, 512)).astype(ml_dtypes.bfloat16),
            "masks": maskv.astype(ml_dtypes.bfloat16),
            "ones": np.ones((P, HPC), np.float32),
        })
    return maps


def kernel(x, Wq, bq, Wk, bk, Wv, bv, Wp, bp, _trace=False):
    if "nc" not in _CACHE:
        _CACHE["nc"] = _build()
    nc = _CACHE["nc"]
    res = run_bass_kernel_spmd(nc, _in_maps(x, Wq, bq, Wk, bk, Wv, bv, Wp, bp),
                               list(range(8)), trace=_trace)
    _CACHE["last_result"] = res
    out = np.empty((4, T, E), np.float32)
    for b in range(4):
        out[b] = res.results[2 * b]["out"] + res.results[2 * b + 1]["out"] + bp
    return out


# revision 21
# speedup vs baseline: 1.5797x; 1.2071x over previous
"""Causal self-attention (B=4, T=2048, E=1024, H=16, D=64) on 8 TRN2 NeuronCores.

Sharding: core c -> batch b=c//2, head-group g=c%2 (8 heads each).
Each core:
  phase A: qT,kT = (Wq_g @ x_b.T), (Wk_g @ x_b.T)      feature-major [512, 2048]
  phase B: v = x_b @ Wv_g.T                             token-major, +ones column
  phase C: per head: S^T = k^T q (keys on partitions), exp via ScalarE,
           causal mask as 0/1 multiply, AV matmul with ones-column folding the
           softmax denominator into partition 64, normalize via reciprocal +
           gpsimd partition_broadcast.
  phase D: partial out = concat(heads_g) @ Wp_g.T       token-major [2048, 1024]
Host: out[b] = partial[2b] + partial[2b+1] + bp.

All matmuls run in float32r (full PE rate, ~1e-4 precision); fp32 elsewhere.
"""
import sys

if '/opt/trn_rl_repo' not in sys.path:
    sys.path.insert(0, '/opt/trn_rl_repo')

from contextlib import ExitStack

import numpy as np
import ml_dtypes

import concourse.bass as bass
import concourse.tile as tile
from concourse import bacc, mybir
from concourse.bass_utils import run_bass_kernel_spmd

P = 128
T = 2048          # tokens per core (one batch)
E = 1024          # embed
HPC = 8           # heads per core
D = 64            # head dim
GD = HPC * D      # 512 group dims per core
NB = T // 512     # 4 query/token blocks of 512
NE = E // P       # 8 contraction chunks
NT = T // P       # 16 token blocks of 128
SLOT = D + 1      # v slot width: [v(64) | ones]
F32 = mybir.dt.float32
FPR = mybir.dt.float32r
EXP = mybir.ActivationFunctionType.Exp

_CACHE = {}


def _build():
    nc = bacc.Bacc("TRN2", target_bir_lowering=False, debug=False)

    xt = nc.dram_tensor("xt", [E, T], FPR, kind="ExternalInput").ap()
    wq = nc.dram_tensor("wq", [E, GD], FPR, kind="ExternalInput").ap()
    wk = nc.dram_tensor("wk", [E, GD], FPR, kind="ExternalInput").ap()
    wv = nc.dram_tensor("wv", [E, GD], FPR, kind="ExternalInput").ap()
    wp = nc.dram_tensor("wp", [GD, E], FPR, kind="ExternalInput").ap()
    bqk = nc.dram_tensor("bqk", [2, P, 4], F32, kind="ExternalInput").ap()
    bvb = nc.dram_tensor("bvb", [P, GD], mybir.dt.bfloat16, kind="ExternalInput").ap()
    masks = nc.dram_tensor("masks", [P, P], mybir.dt.bfloat16, kind="ExternalInput").ap()
    ones = nc.dram_tensor("ones", [P, HPC], FPR, kind="ExternalInput").ap()
    out = nc.dram_tensor("out", [T, E], F32, kind="ExternalOutput").ap()

    xtr = xt.rearrange("(eo ep) t -> ep eo t", ep=P)      # [128, 8, 2048]
    wqr = wq.rearrange("(eo ep) m -> ep eo m", ep=P)      # [128, 8, 512]
    wkr = wk.rearrange("(eo ep) m -> ep eo m", ep=P)
    wvr = wv.rearrange("(eo ep) m -> ep eo m", ep=P)
    wpr = wp.rearrange("(co cp) o -> cp co o", cp=P)      # [128, 4, 1024]
    outr = out.rearrange("(to tp) o -> to tp o", tp=P)    # [16, 128, 1024]

    with tile.TileContext(nc) as tc, ExitStack() as ctx:
        # ---- persistent pools ----
        qk_pool = ctx.enter_context(tc.tile_pool(name="qk", bufs=1))
        vaug_pool = ctx.enter_context(tc.tile_pool(name="vaug", bufs=1))
        ho_pool = ctx.enter_context(tc.tile_pool(name="ho", bufs=1))
        const_pool = ctx.enter_context(tc.tile_pool(name="const", bufs=1))
        exp_pool = ctx.enter_context(tc.tile_pool(name="exps", bufs=4))
        norm_pool = ctx.enter_context(tc.tile_pool(name="norm", bufs=2))
        out_pool = ctx.enter_context(tc.tile_pool(name="outs", bufs=2))

        qT = [qk_pool.tile([P, T], FPR, tag=f"qT{i}", name=f"qT{i}") for i in range(4)]
        kT = [qk_pool.tile([P, T], FPR, tag=f"kT{i}", name=f"kT{i}") for i in range(4)]
        vaug = [vaug_pool.tile([P, HPC * SLOT], FPR, tag=f"va{t}", name=f"va{t}")
                for t in range(NT)]
        hoT = [ho_pool.tile([P, T], FPR, tag=f"hoT{i}", name=f"hoT{i}") for i in range(4)]

        bqk_t = const_pool.tile([P, 2, 4], F32, tag="bqk", name="bqk_t")
        nc.sync.dma_start(bqk_t[:], bqk.rearrange("b p m -> p b m"))
        bvb_t = const_pool.tile([P, GD], mybir.dt.bfloat16, tag="bvb", name="bvb_t")
        nc.sync.dma_start(bvb_t[:], bvb)
        masks_t = const_pool.tile([P, P], mybir.dt.bfloat16, tag="masks", name="masks_t")
        nc.sync.dma_start(masks_t[:], masks)
        onec_t = const_pool.tile([P, HPC], FPR, tag="onec", name="onec_t")
        nc.sync.dma_start(onec_t[:], ones)

        # ---- phases A+B: q,k,v projections. Weights are STREAMED (each chunk
        # is only reused 4x; residency buys nothing and costs 48KB/partition).
        with ExitStack() as actx:
            xs_pool = actx.enter_context(tc.tile_pool(name="xs", bufs=3))
            ws_pool = actx.enter_context(tc.tile_pool(name="ws", bufs=4))
            psA = actx.enter_context(tc.tile_pool(name="psA", bufs=4, space="PSUM"))

            for n in range(NB):
                nsl = bass.ts(n, 512)
                qps = [psA.tile([P, 512], F32, tag="qps", name=f"qps{n}_{m}")
                       for m in range(4)]
                kps = [psA.tile([P, 512], F32, tag="kps", name=f"kps{n}_{m}")
                       for m in range(4)]
                for eh in range(NE // 2):
                    xs = xs_pool.tile([P, 2, 512], FPR, tag="xs", name=f"xsA{n}_{eh}")
                    nc.sync.dma_start(xs[:], xtr[:, 2 * eh:2 * eh + 2, nsl])
                    wqs = ws_pool.tile([P, 2, 512], FPR, tag="ws", name=f"wqs{n}_{eh}")
                    nc.sync.dma_start(wqs[:], wqr[:, 2 * eh:2 * eh + 2, :])
                    wks = ws_pool.tile([P, 2, 512], FPR, tag="ws", name=f"wks{n}_{eh}")
                    nc.sync.dma_start(wks[:], wkr[:, 2 * eh:2 * eh + 2, :])
                    for i in range(2):
                        e = 2 * eh + i
                        st = (e == 0)
                        sp = (e == NE - 1)
                        xmov = xs[:, i]
                        for m in range(4):
                            nc.tensor.matmul(qps[m][:], wqs[:, i, bass.ts(m, P)],
                                             xmov, start=st, stop=sp)
                        for m in range(4):
                            nc.tensor.matmul(kps[m][:], wks[:, i, bass.ts(m, P)],
                                             xmov, start=st, stop=sp)
                for m in range(4):
                    nc.vector.tensor_scalar_add(qT[m][:, nsl], qps[m][:], bqk_t[:, 0, m:m + 1])
                for m in range(4):
                    nc.vector.tensor_scalar_add(kT[m][:, nsl], kps[m][:], bqk_t[:, 1, m:m + 1])

            # ---- phase B: v projection (token-major, with ones column) ----
            for tg in range(4):
                vps = [psA.tile([P, GD], F32, tag=("qps" if jj < 2 else "kps"),
                                name=f"vps{tg}_{jj}")
                       for jj in range(4)]
                for ep in range(4):
                    xbt = xs_pool.tile([P, 2, 512], FPR, tag="xs", name=f"xb{tg}_{ep}")
                    nc.sync.dma_start(xbt[:], xtr[:, 2 * ep:2 * ep + 2, bass.ts(tg, 512)])
                    wvs = ws_pool.tile([P, 2, 512], FPR, tag="ws", name=f"wvs{tg}_{ep}")
                    nc.sync.dma_start(wvs[:], wvr[:, 2 * ep:2 * ep + 2, :])
                    for i in range(2):
                        e = 2 * ep + i
                        for jj in range(4):
                            nc.tensor.matmul(vps[jj][:], xbt[:, i, bass.ts(jj, P)],
                                             wvs[:, i],
                                             start=(e == 0), stop=(e == NE - 1))
                for jj in range(4):
                    t = 4 * tg + jj
                    nc.vector.tensor_copy(
                        vaug[t][:].rearrange("p (h s) -> p h s", s=SLOT)[:, :, D:D + 1],
                        onec_t[:].unsqueeze(2))
                    nc.vector.tensor_tensor(
                        vaug[t][:].rearrange("p (h s) -> p h s", s=SLOT)[:, :, 0:D],
                        vps[jj][:].rearrange("p (h d) -> p h d", d=D),
                        bvb_t[:].rearrange("p (h d) -> p h d", d=D),
                        mybir.AluOpType.add)

        # ---- phase C: attention per head-pair ----
        with ExitStack() as cctx:
            psC = cctx.enter_context(tc.tile_pool(name="psC", bufs=3, space="PSUM"))
            psAV = cctx.enter_context(tc.tile_pool(name="psAV", bufs=1, space="PSUM"))

            for hp in range(4):
                slotA = bass.ds((2 * hp) * SLOT, SLOT)
                slotB = bass.ds((2 * hp + 1) * SLOT, SLOT)
                for qb in range(NB):
                    qsl0 = qb * 512
                    nkb = 4 * (qb + 1)
                    avA = psAV.tile([P, 512], F32, tag="avA", name=f"avA{hp}_{qb}")
                    avB = psAV.tile([P, 512], F32, tag="avB", name=f"avB{hp}_{qb}")

                    def emit_s(kb, hp=hp, qb=qb, qsl0=qsl0):
                        # straddle tile j: columns f < 128j are fully masked ->
                        # compute only columns [q0:512]; the triangle mask only
                        # touches the diagonal 128-column block [q0:q0+128).
                        ksl = bass.ts(kb, P)
                        j = kb - 4 * qb
                        q0 = 128 * j if j > 0 else 0
                        sp = psC.tile([P, 1024], F32, tag="sps", name=f"sps{hp}_{qb}_{kb}")
                        et = exp_pool.tile([P, 1024], FPR, tag="expS", name=f"ex{hp}_{qb}_{kb}")
                        for half, pl, ph in ((0, 0, D), (1, D, P)):
                            o = 512 * half
                            nc.tensor.matmul(sp[:, o + q0:o + 512], kT[hp][pl:ph, ksl],
                                             qT[hp][pl:ph, qsl0 + q0:qsl0 + 512],
                                             start=True, stop=True)
                        if j < 0 or q0 == 0:
                            nc.scalar.activation(et[:], sp[:], EXP, scale=0.125)
                        else:
                            nc.scalar.activation(et[:, q0:512], sp[:, q0:512],
                                                 EXP, scale=0.125)
                            nc.scalar.activation(et[:, 512 + q0:1024], sp[:, 512 + q0:1024],
                                                 EXP, scale=0.125)
                        if j >= 0:
                            dsl = bass.ds(q0, P)
                            nc.vector.tensor_tensor(
                                et[:].rearrange("p (two n) -> p two n", two=2)[:, :, dsl],
                                et[:].rearrange("p (two n) -> p two n", two=2)[:, :, dsl],
                                masks_t[:, None, :].to_broadcast([P, 2, P]),
                                mybir.AluOpType.mult)
                        return et, q0

                    pend = {0: emit_s(0)}
                    for kb in range(nkb):
                        if kb + 1 < nkb:
                            pend[kb + 1] = emit_s(kb + 1)
                        et, q0 = pend.pop(kb)
                        st = (kb == 0)
                        sp_ = (kb == nkb - 1)
                        nc.tensor.matmul(avA[0:SLOT, q0:512], vaug[kb][:, slotA],
                                         et[:, q0:512], start=st, stop=sp_)
                        nc.tensor.matmul(avB[0:SLOT, q0:512], vaug[kb][:, slotB],
                                         et[:, 512 + q0:1024], start=st, stop=sp_)

                    # normalize: out[d,q] / denom[q]; denom at partition 64 (ones
                    # col last in v slots). Evacuate av psum to SBUF immediately so
                    # the psum bank frees fast (keeps PE dense / HAM warm), then
                    # reciprocal on a [32,32] reshape (32 lanes), broadcast from
                    # physical partition 0, multiply.
                    qsl = bass.ds(qsl0, 512)
                    avSA = norm_pool.tile([SLOT, 512], F32, tag="avSA", name=f"avSA{hp}_{qb}")
                    nc.vector.tensor_copy(avSA[:], avA[0:SLOT, :])
                    avSB = norm_pool.tile([SLOT, 512], F32, tag="avSB", name=f"avSB{hp}_{qb}")
                    nc.vector.tensor_copy(avSB[:], avB[0:SLOT, :])
                    d32 = norm_pool.tile([32, 32], F32, tag="d32", name=f"d32{hp}_{qb}")
                    nc.sync.dma_start(d32[:, 0:16], avSA[D:SLOT, :])
                    nc.sync.dma_start(d32[:, 16:32], avSB[D:SLOT, :])
                    nc.vector.reciprocal(d32[:], d32[:])
                    rc0A = norm_pool.tile([1, 512], F32, tag="rc0A", name=f"rc0A{hp}_{qb}")
                    nc.sync.dma_start(rc0A[0:1, :], d32[:, 0:16])
                    rc0B = norm_pool.tile([1, 512], F32, tag="rc0B", name=f"rc0B{hp}_{qb}")
                    nc.sync.dma_start(rc0B[0:1, :], d32[:, 16:32])
                    bcA = norm_pool.tile([D, 512], F32, tag="bcA", name=f"bcA{hp}_{qb}")
                    nc.gpsimd.partition_broadcast(bcA[:], rc0A[0:1, :], channels=D)
                    bcB = norm_pool.tile([D, 512], F32, tag="bcB", name=f"bcB{hp}_{qb}")
                    nc.gpsimd.partition_broadcast(bcB[:], rc0B[0:1, :], channels=D)
                    nc.vector.tensor_tensor(hoT[hp][0:D, qsl], avSA[0:D, :], bcA[:],
                                            mybir.AluOpType.mult)
                    tmpB = norm_pool.tile([D, 512], FPR, tag="tmpB", name=f"tmpB{hp}_{qb}")
                    nc.vector.tensor_tensor(tmpB[:], avSB[0:D, :], bcB[:],
                                            mybir.AluOpType.mult)
                    nc.sync.dma_start(hoT[hp][D:P, qsl], tmpB[:])

        # ---- phase D: output projection (token-major partial) ----
        with ExitStack() as dctx:
            wp_pool = dctx.enter_context(tc.tile_pool(name="wpp", bufs=1))
            psD = dctx.enter_context(tc.tile_pool(name="psD", bufs=4, space="PSUM"))
            wp_t = wp_pool.tile([P, 4, E], FPR, tag="wp", name="wp_t")
            nc.sync.dma_start(wp_t[:], wpr)

            for t in range(NT):
                tsl = bass.ts(t, P)
                for nh in range(2):
                    dps = psD.tile([P, 512], F32, tag="dps", name=f"dps{t}_{nh}")
                    for c in range(4):
                        nc.tensor.matmul(dps[:], hoT[c][:, tsl],
                                         wp_t[:, c, bass.ts(nh, 512)],
                                         start=(c == 0), stop=(c == 3))
                    ot = out_pool.tile([P, 512], F32, tag="ot", name=f"ot{t}_{nh}")
                    nc.vector.tensor_copy(ot[:], dps[:])
                    nc.sync.dma_start(outr[t, :, bass.ts(nh, 512)], ot[:])

    nc.compile()
    return nc


def _in_maps(x, Wq, bq, Wk, bk, Wv, bv, Wp, bp):
    maskv = (np.arange(P)[:, None] <= np.arange(P)[None, :]).astype(np.float32)
    maps = []
    for c in range(8):
        b, g = divmod(c, 2)
        gs = slice(512 * g, 512 * (g + 1))
        maps.append({
            "xt": np.ascontiguousarray(x[b].T),
            "wq": np.ascontiguousarray(Wq[gs, :].T),
            "wk": np.ascontiguousarray(Wk[gs, :].T),
            "wv": np.ascontiguousarray(Wv[gs, :].T),
            "wp": np.ascontiguousarray(Wp[:, gs].T),
            "bqk": np.stack([bq[gs].reshape(4, P).T, bk[gs].reshape(4, P).T]),
            "bvb": np.broadcast_to(bv[gs], (P, 512)).astype(ml_dtypes.bfloat16),
            "masks": maskv.astype(ml_dtypes.bfloat16),
            "ones": np.ones((P, HPC), np.float32),
        })
    return maps


def kernel(x, Wq, bq, Wk, bk, Wv, bv, Wp, bp, _trace=False):
    if "nc" not in _CACHE:
        _CACHE["nc"] = _build()
    nc = _CACHE["nc"]
    res = run_bass_kernel_spmd(nc, _in_maps(x, Wq, bq, Wk, bk, Wv, bv, Wp, bp),
                               list(range(8)), trace=_trace)
    _CACHE["last_result"] = res
    out = np.empty((4, T, E), np.float32)
    for b in range(4):
        out[b] = res.results[2 * b]["out"] + res.results[2 * b + 1]["out"] + bp
    return out
